# revision 3
# baseline (speedup 1.0000x reference)
"""Distributed Trainium2 kernel for GPT-2 style multi-head causal attention.

reference:
    qkv = x @ w_attn + b_attn            # [B,S,3*NX]
    q,k,v split; 16 heads, DH=64; causal softmax(q k^T / sqrt(DH)) v
    out = a @ w_proj + b_proj            # [B,S,NX]

Sharding over 8 NeuronCores: core c -> (batch b=c//2, head-group g=c%2).
Each core computes qkv for its batch and its 8 heads (Megatron column-parallel
c_attn), flash attention for those 8 heads fully in SBUF, then a 2-core
AllGather of the per-head attention outputs within each batch pair, and a
column-parallel c_proj (each core produces 512 of the 1024 output features
for all 2048 tokens of its batch). Host concatenates.

Compute in bf16 on the TensorEngine with fp32 PSUM accumulation; softmax is
the "unsafe" variant (no row-max subtraction) which is exact here: score
magnitudes are O(1) and masked lanes are multiplied by an exact 0/1 mask
after exp.
"""

import sys

if "/opt/trn_rl_repo" not in sys.path:
    sys.path.insert(0, "/opt/trn_rl_repo")

import numpy as np
import ml_dtypes

import concourse.bass as bass
import concourse.mybir as mybir
import concourse.tile as tile
from concourse import bacc
from concourse.bass_utils import run_bass_kernel_spmd

BF16 = ml_dtypes.bfloat16

B, S, NX, H = 4, 2048, 1024, 16
DH = NX // H  # 64
N_CORES = 8
HPC = 8          # heads per core
FQK = HPC * DH   # 512 q (or k) features per core
GQ = S // 512    # 4 q-tiles of 512
KC = S // 128    # 16 k-chunks of 128
TT16 = S // 128  # 16 token chunks of 128

f32 = mybir.dt.float32
bf16 = mybir.dt.bfloat16

_BUILD_CACHE: dict = {}


def build_nc(debug_taps: bool = False):
    """Build + compile the SPMD Bass graph (identical on all 8 cores)."""
    key = ("nc", debug_taps)
    if key in _BUILD_CACHE:
        return _BUILD_CACHE[key]

    nc = bacc.Bacc("TRN2", target_bir_lowering=False, debug=False, num_devices=N_CORES)

    xT = nc.dram_tensor("xT", [NX, S], bf16, kind="ExternalInput")
    wqk = nc.dram_tensor("wqk", [NX, 2 * FQK], bf16, kind="ExternalInput")
    wv = nc.dram_tensor("wv", [NX, FQK], bf16, kind="ExternalInput")
    bqk = nc.dram_tensor("bqk", [128, 8], f32, kind="ExternalInput")
    bvb = nc.dram_tensor("bvb", [128, FQK], f32, kind="ExternalInput")
    wpj = nc.dram_tensor("wpj", [NX, FQK], bf16, kind="ExternalInput")
    bpj = nc.dram_tensor("bpj", [128, FQK], f32, kind="ExternalInput")
    msk = nc.dram_tensor("msk", [128, 4, 512], bf16, kind="ExternalInput")
    out = nc.dram_tensor("out", [S, FQK], f32, kind="ExternalOutput")
    if debug_taps:
        qkT_tap = nc.dram_tensor("qkT_tap", [128, 8, S], f32, kind="ExternalOutput")
        von_tap = nc.dram_tensor("von_tap", [128, TT16, HPC, 65], f32, kind="ExternalOutput")
        aT_tap = nc.dram_tensor("aT_tap", [128, 4, S], f32, kind="ExternalOutput")

    # internal DRAM for the collective
    ag_in = nc.dram_tensor("ag_in", [FQK, S], bf16)
    ag_out = nc.dram_tensor("ag_out", [NX, S], bf16)

    groups = [[0, 1], [2, 3], [4, 5], [6, 7]]

    with tile.TileContext(nc) as tc:
        with (
            tc.tile_pool(name="persist", bufs=1) as pp,
            tc.tile_pool(name="ptmp", bufs=3) as ptmp,
            tc.tile_pool(name="oevict", bufs=3) as oev,
            tc.tile_pool(name="ps_s", bufs=3, space="PSUM") as ps_s,
            tc.tile_pool(name="ps_a", bufs=2, space="PSUM") as ps_a,
            tc.tile_pool(name="ps_r", bufs=1, space="PSUM") as ps_r,
        ):
            # ---- persistent SBUF tensors
            qkT = pp.tile([128, 8, S], bf16)       # fc 0-3: q, 4-7: k (2 heads/chunk)
            von = pp.tile([128, TT16, HPC, 65], bf16)  # v natural + ones column
            aT = pp.tile([128, 4, S], bf16)        # per-head attn out (f' x t)
            msk_sb = pp.tile([128, 4, 512], bf16)
            bqk_sb = pp.tile([128, 8], f32)
            bvb_sb = pp.tile([128, FQK], f32)
            bpj_sb = pp.tile([128, FQK], f32)
            wpj_sb = pp.tile([128, 8, FQK], bf16)
            ones1 = pp.tile([1, 64], f32)
            zb = pp.tile([128, 1], f32)

            nc.sync.dma_start(msk_sb[:], msk[:])
            nc.sync.dma_start(bqk_sb[:], bqk[:])
            nc.sync.dma_start(bvb_sb[:], bvb[:])
            nc.sync.dma_start(bpj_sb[:], bpj[:])
            nc.sync.dma_start(wpj_sb[:], wpj[:].rearrange("(c p) f -> p c f", p=128))
            nc.vector.memset(ones1[:], 1.0)
            nc.vector.memset(zb[:], 0.0)
            # ones column of von (softmax denominator trick)
            nc.vector.memset(von[:, :, :, 64:65], 1.0)

            with (
                tc.tile_pool(name="g1", bufs=1) as g1p,
                tc.tile_pool(name="ps_g1", bufs=2, space="PSUM") as ps_g1,
            ):
                xT_sb = g1p.tile([128, 8, S], bf16)
                wqk_sb = g1p.tile([128, 8, 2 * FQK], bf16)
                wv_sb = g1p.tile([128, 8, FQK], bf16)
                # split input DMAs so compute can start early
                for q in range(4):
                    sl = slice(q * (S // 4), (q + 1) * (S // 4))
                    nc.sync.dma_start(
                        xT_sb[:, :, sl],
                        xT[:, sl].rearrange("(c p) t -> p c t", p=128),
                    )
                nc.sync.dma_start(
                    wqk_sb[:], wqk[:].rearrange("(c p) f -> p c f", p=128)
                )
                nc.sync.dma_start(wv_sb[:], wv[:].rearrange("(c p) f -> p c f", p=128))

                # ---- GEMM1b: v natural layout [t, h, d] (+ones col kept intact)
                for tt in range(TT16):
                    ps = ps_g1.tile([128, 512], f32, tag="g1")
                    for c in range(8):
                        nc.tensor.matmul(
                            ps[:],
                            xT_sb[:, c, tt * 128 : (tt + 1) * 128],
                            wv_sb[:, c, :],
                            start=(c == 0),
                            stop=(c == 7),
                        )
                        # evict with bias -> von[:, tt, :, 0:64]
                    nc.vector.tensor_tensor(
                        von[:, tt, :, 0:64],
                        ps[:].rearrange("p (h d) -> p h d", d=64),
                        bvb_sb[:].rearrange("p (h d) -> p h d", d=64),
                        mybir.AluOpType.add,
                    )

                # ---- GEMM1a (q,k transposed layout) interleaved with flash
                def gemm1a_chunk(fc):
                    for tt in range(GQ):
                        ps = ps_g1.tile([128, 512], f32, tag="g1")
                        for c in range(8):
                            nc.tensor.matmul(
                                ps[:],
                                wqk_sb[:, c, fc * 128 : (fc + 1) * 128],
                                xT_sb[:, c, tt * 512 : (tt + 1) * 512],
                                start=(c == 0),
                                stop=(c == 7),
                            )
                        nc.vector.tensor_scalar(
                            qkT[:, fc, tt * 512 : (tt + 1) * 512],
                            ps[:],
                            bqk_sb[:, fc : fc + 1],
                            None,
                            mybir.AluOpType.add,
                        )

                def flash_pair(i):
                    # heads 2i (partitions 0-63) and 2i+1 (partitions 64-127)
                    for qt in range(GQ):
                        nkc = 4 * (qt + 1)
                        a_ps = [
                            ps_a.tile([65, 512], f32, tag="aT", name=f"aps{i}_{qt}_{h}")
                            for h in range(2)
                        ]
                        for kc in range(nkc):
                            j = kc - 4 * qt  # >=0 -> diagonal-overlap chunk
                            for h in range(2):
                                p0 = 64 * h
                                sT = ps_s.tile(
                                    [128, 512], f32, tag="sT", name=f"sT{i}_{qt}_{kc}_{h}"
                                )
                                nc.tensor.matmul(
                                    sT[:],
                                    qkT[p0 : p0 + 64, 4 + i, kc * 128 : (kc + 1) * 128],
                                    qkT[p0 : p0 + 64, i, qt * 512 : (qt + 1) * 512],
                                    start=True,
                                    stop=True,
                                )
                                pT = ptmp.tile(
                                    [128, 512], bf16, tag="pT", name=f"pT{i}_{qt}_{kc}_{h}"
                                )
                                nc.scalar.activation(
                                    pT[:],
                                    sT[:],
                                    mybir.ActivationFunctionType.Exp,
                                    bias=zb[:],
                                    scale=0.125,
                                )
                                if j >= 0:
                                    nc.vector.tensor_tensor(
                                        pT[:],
                                        pT[:],
                                        msk_sb[:, j, :],
                                        mybir.AluOpType.mult,
                                    )
                                nc.tensor.matmul(
                                    a_ps[h][:],
                                    von[:, kc, 2 * i + h, :],
                                    pT[:],
                                    start=(kc == 0),
                                    stop=(kc == nkc - 1),
                                )
                        for h in range(2):
                            rec = ptmp.tile([1, 512], f32, tag="rec")
                            nc.vector.reciprocal(rec[:], a_ps[h][64:65, :])
                            rb = ps_r.tile([64, 512], f32, tag="rb")
                            nc.tensor.matmul(
                                rb[:], ones1[:], rec[:], start=True, stop=True
                            )
                            rb_sb = ptmp.tile([64, 512], f32, tag="rbs")
                            nc.vector.tensor_copy(rb_sb[:], rb[:])
                            nc.vector.tensor_tensor(
                                aT[64 * h : 64 * h + 64, i, qt * 512 : (qt + 1) * 512],
                                a_ps[h][0:64, :],
                                rb_sb[:],
                                mybir.AluOpType.mult,
                            )

                for i in range(4):
                    gemm1a_chunk(i)
                    gemm1a_chunk(4 + i)
                    flash_pair(i)

            # ---- AllGather of aT across the 2-core batch pair
            nc.sync.dma_start(
                ag_in[:].rearrange("(fc p) t -> p fc t", p=128), aT[:]
            )
            nc.gpsimd.collective_compute(
                "AllGather",
                mybir.AluOpType.bypass,
                replica_groups=groups,
                ins=[ag_in[:].opt()],
                outs=[ag_out[:].opt()],
            )

            with (
                tc.tile_pool(name="g2", bufs=1) as g2p,
                tc.tile_pool(name="ps_g2", bufs=2, space="PSUM") as ps_g2,
            ):
                gath = g2p.tile([128, 8, S], bf16)
                nc.sync.dma_start(
                    gath[:], ag_out[:].rearrange("(c p) t -> p c t", p=128)
                )
                for tt in range(TT16):
                    ps = ps_g2.tile([128, 512], f32, tag="g2")
                    for c in range(8):
                        nc.tensor.matmul(
                            ps[:],
                            gath[:, c, tt * 128 : (tt + 1) * 128],
                            wpj_sb[:, c, :],
                            start=(c == 0),
                            stop=(c == 7),
                        )
                    og = oev.tile([128, 512], f32, tag="og")
                    nc.vector.tensor_tensor(
                        og[:], ps[:], bpj_sb[:], mybir.AluOpType.add
                    )
                    nc.sync.dma_start(out[tt * 128 : (tt + 1) * 128, :], og[:])

            if debug_taps:
                tq = oev.tile([128, 8, S], f32, tag="tapq")
                nc.vector.tensor_copy(tq[:], qkT[:])
                nc.sync.dma_start(qkT_tap[:], tq[:])
                tv = oev.tile([128, TT16, HPC, 65], f32, tag="tapv")
                nc.vector.tensor_copy(tv[:], von[:])
                nc.sync.dma_start(von_tap[:], tv[:])
                ta = oev.tile([128, 4, S], f32, tag="tapa")
                nc.vector.tensor_copy(ta[:], aT[:])
                nc.sync.dma_start(aT_tap[:], ta[:])

    nc.compile()
    _BUILD_CACHE[key] = nc
    return nc


def make_in_maps(x, w_attn, b_attn, w_proj, b_proj):
    """Shard the full inputs into 8 per-core input maps."""
    x = np.asarray(x, dtype=np.float32)
    w_attn = np.asarray(w_attn, dtype=np.float32)
    b_attn = np.asarray(b_attn, dtype=np.float32)
    w_proj = np.asarray(w_proj, dtype=np.float32)
    b_proj = np.asarray(b_proj, dtype=np.float32)

    kp = np.arange(128)[:, None, None]
    jj = np.arange(4)[None, :, None]
    qf = np.arange(512)[None, None, :]
    mask = (kp + 128 * jj <= qf).astype(BF16)

    in_maps = []
    for c in range(N_CORES):
        b, g = c // 2, c % 2
        sl = slice(g * FQK, (g + 1) * FQK)
        wq = w_attn[:, 0 * NX :][:, sl]
        wk = w_attn[:, 1 * NX :][:, sl]
        wv_ = w_attn[:, 2 * NX :][:, sl]
        bq = b_attn[0 * NX :][sl]
        bk = b_attn[1 * NX :][sl]
        bv_ = b_attn[2 * NX :][sl]
        in_maps.append(
            {
                "xT": np.ascontiguousarray(x[b].T).astype(BF16),
                "wqk": np.ascontiguousarray(
                    np.concatenate([wq, wk], axis=1)
                ).astype(BF16),
                "wv": np.ascontiguousarray(wv_).astype(BF16),
                "bqk": np.ascontiguousarray(
                    np.concatenate([bq, bk]).reshape(8, 128).T
                ).astype(np.float32),
                "bvb": np.ascontiguousarray(
                    np.broadcast_to(bv_[None, :], (128, FQK))
                ).astype(np.float32),
                "wpj": np.ascontiguousarray(w_proj[:, sl]).astype(BF16),
                "bpj": np.ascontiguousarray(
                    np.broadcast_to(b_proj[None, sl], (128, FQK))
                ).astype(np.float32),
                "msk": mask,
            }
        )
    return in_maps


def assemble_out(results):
    out = np.empty((B, S, NX), dtype=np.float32)
    for c in range(N_CORES):
        b, g = c // 2, c % 2
        out[b, :, g * FQK : (g + 1) * FQK] = results[c]["out"]
    return out


def kernel(x, w_attn, b_attn, w_proj, b_proj):
    nc = build_nc()
    in_maps = make_in_maps(x, w_attn, b_attn, w_proj, b_proj)
    res = run_bass_kernel_spmd(nc, in_maps, core_ids=list(range(N_CORES)))
    return assemble_out(res.results)


# revision 5
# speedup vs baseline: 772.2497x; 772.2497x over previous
"""Distributed Trainium2 kernel for GPT-2 style multi-head causal attention.

reference:
    qkv = x @ w_attn + b_attn            # [B,S,3*NX]
    q,k,v split; 16 heads, DH=64; causal softmax(q k^T / sqrt(DH)) v
    out = a @ w_proj + b_proj            # [B,S,NX]

Sharding over 8 NeuronCores: core c -> (batch b=c//2, head-group g=c%2).
Each core computes qkv for its batch and its 8 heads (Megatron column-parallel
c_attn), flash attention for those 8 heads fully in SBUF, then a 2-core
AllGather of the per-head attention outputs within each batch pair, and a
column-parallel c_proj (each core produces 512 of the 1024 output features
for all 2048 tokens of its batch). Host concatenates.

Compute in bf16 on the TensorEngine with fp32 PSUM accumulation; softmax is
the "unsafe" variant (no row-max subtraction) which is exact here: score
magnitudes are O(1) and masked lanes are multiplied by an exact 0/1 mask
after exp.
"""

import sys

if "/opt/trn_rl_repo" not in sys.path:
    sys.path.insert(0, "/opt/trn_rl_repo")

import numpy as np
import ml_dtypes

import concourse.bass as bass
import concourse.mybir as mybir
import concourse.tile as tile
from concourse import bacc
from concourse.bass_utils import run_bass_kernel_spmd

BF16 = ml_dtypes.bfloat16

B, S, NX, H = 4, 2048, 1024, 16
DH = NX // H  # 64
N_CORES = 8
HPC = 8          # heads per core
FQK = HPC * DH   # 512 q (or k) features per core
GQ = S // 512    # 4 q-tiles of 512
KC = S // 128    # 16 k-chunks of 128
TT16 = S // 128  # 16 token chunks of 128

f32 = mybir.dt.float32
bf16 = mybir.dt.bfloat16

_BUILD_CACHE: dict = {}


def build_nc(debug_taps: bool = False, reps: int = 1, sim_single: bool = False):
    """Build + compile the SPMD Bass graph (identical on all 8 cores).

    reps>1 replicates the whole body (for slope-based timing: the axon
    dispatch overhead is large, so per-exec time = slope of wall vs reps).
    sim_single builds a 1-core variant with the collective replaced by
    equivalent local DMAs, for TimelineSim cost-model profiling.
    """
    key = ("nc", debug_taps, reps, sim_single)
    if key in _BUILD_CACHE:
        return _BUILD_CACHE[key]

    ndev = 1 if sim_single else N_CORES
    nc = bacc.Bacc("TRN2", target_bir_lowering=False, debug=False, num_devices=ndev)

    xT = nc.dram_tensor("xT", [NX, S], bf16, kind="ExternalInput")
    wqk = nc.dram_tensor("wqk", [NX, 2 * FQK], bf16, kind="ExternalInput")
    wv = nc.dram_tensor("wv", [NX, FQK], bf16, kind="ExternalInput")
    bqk = nc.dram_tensor("bqk", [128, 8], f32, kind="ExternalInput")
    bvb = nc.dram_tensor("bvb", [128, FQK], f32, kind="ExternalInput")
    wpj = nc.dram_tensor("wpj", [NX, FQK], bf16, kind="ExternalInput")
    bpj = nc.dram_tensor("bpj", [128, FQK], f32, kind="ExternalInput")
    msk = nc.dram_tensor("msk", [128, 4, 512], bf16, kind="ExternalInput")
    out = nc.dram_tensor("out", [S, FQK], f32, kind="ExternalOutput")
    if debug_taps:
        qkT_tap = nc.dram_tensor("qkT_tap", [128, 8, S], f32, kind="ExternalOutput")
        von_tap = nc.dram_tensor("von_tap", [128, TT16, HPC, 65], f32, kind="ExternalOutput")
        aT_tap = nc.dram_tensor("aT_tap", [128, 4, S], f32, kind="ExternalOutput")

    groups = [[0, 1], [2, 3], [4, 5], [6, 7]]

    with tile.TileContext(nc) as tc:
      for _rep in range(reps):
        # internal DRAM for the collective
        ag_in = nc.dram_tensor(f"ag_in{_rep}", [FQK, S], bf16)
        ag_out = nc.dram_tensor(f"ag_out{_rep}", [NX, S], bf16)
        with (
            tc.tile_pool(name="persist", bufs=1) as pp,
            tc.tile_pool(name="ptmp", bufs=3) as ptmp,
            tc.tile_pool(name="oevict", bufs=3) as oev,
            tc.tile_pool(name="ps_s", bufs=3, space="PSUM") as ps_s,
            tc.tile_pool(name="ps_a", bufs=2, space="PSUM") as ps_a,
            tc.tile_pool(name="ps_r", bufs=1, space="PSUM") as ps_r,
        ):
            # ---- persistent SBUF tensors
            qkT = pp.tile([128, 8, S], bf16)       # fc 0-3: q, 4-7: k (2 heads/chunk)
            von = pp.tile([128, TT16, HPC, 65], bf16)  # v natural + ones column
            aT = pp.tile([128, 4, S], bf16)        # per-head attn out (f' x t)
            msk_sb = pp.tile([128, 4, 512], bf16)
            bqk_sb = pp.tile([128, 8], f32)
            bvb_sb = pp.tile([128, FQK], f32)
            bpj_sb = pp.tile([128, FQK], f32)
            wpj_sb = pp.tile([128, 8, FQK], bf16)
            ones1 = pp.tile([1, 64], f32)
            zb = pp.tile([128, 1], f32)

            nc.sync.dma_start(msk_sb[:], msk[:])
            nc.sync.dma_start(bqk_sb[:], bqk[:])
            nc.sync.dma_start(bvb_sb[:], bvb[:])
            nc.sync.dma_start(bpj_sb[:], bpj[:])
            nc.sync.dma_start(wpj_sb[:], wpj[:].rearrange("(c p) f -> p c f", p=128))
            nc.vector.memset(ones1[:], 1.0)
            nc.vector.memset(zb[:], 0.0)
            # ones column of von (softmax denominator trick)
            nc.vector.memset(von[:, :, :, 64:65], 1.0)

            with (
                tc.tile_pool(name="g1", bufs=1) as g1p,
                tc.tile_pool(name="ps_g1", bufs=2, space="PSUM") as ps_g1,
            ):
                xT_sb = g1p.tile([128, 8, S], bf16)
                wqk_sb = g1p.tile([128, 8, 2 * FQK], bf16)
                wv_sb = g1p.tile([128, 8, FQK], bf16)
                # split input DMAs so compute can start early
                for q in range(4):
                    sl = slice(q * (S // 4), (q + 1) * (S // 4))
                    nc.sync.dma_start(
                        xT_sb[:, :, sl],
                        xT[:, sl].rearrange("(c p) t -> p c t", p=128),
                    )
                nc.sync.dma_start(
                    wqk_sb[:], wqk[:].rearrange("(c p) f -> p c f", p=128)
                )
                nc.sync.dma_start(wv_sb[:], wv[:].rearrange("(c p) f -> p c f", p=128))

                # ---- GEMM1b: v natural layout [t, h, d] (+ones col kept intact)
                for tt in range(TT16):
                    ps = ps_g1.tile([128, 512], f32, tag="g1")
                    for c in range(8):
                        nc.tensor.matmul(
                            ps[:],
                            xT_sb[:, c, tt * 128 : (tt + 1) * 128],
                            wv_sb[:, c, :],
                            start=(c == 0),
                            stop=(c == 7),
                        )
                        # evict with bias -> von[:, tt, :, 0:64]
                    nc.vector.tensor_tensor(
                        von[:, tt, :, 0:64],
                        ps[:].rearrange("p (h d) -> p h d", d=64),
                        bvb_sb[:].rearrange("p (h d) -> p h d", d=64),
                        mybir.AluOpType.add,
                    )

                # ---- GEMM1a (q,k transposed layout) interleaved with flash
                def gemm1a_chunk(fc):
                    for tt in range(GQ):
                        ps = ps_g1.tile([128, 512], f32, tag="g1")
                        for c in range(8):
                            nc.tensor.matmul(
                                ps[:],
                                wqk_sb[:, c, fc * 128 : (fc + 1) * 128],
                                xT_sb[:, c, tt * 512 : (tt + 1) * 512],
                                start=(c == 0),
                                stop=(c == 7),
                            )
                        nc.vector.tensor_scalar(
                            qkT[:, fc, tt * 512 : (tt + 1) * 512],
                            ps[:],
                            bqk_sb[:, fc : fc + 1],
                            None,
                            mybir.AluOpType.add,
                        )

                def flash_pair(i):
                    # heads 2i (partitions 0-63) and 2i+1 (partitions 64-127)
                    for qt in range(GQ):
                        nkc = 4 * (qt + 1)
                        a_ps = [
                            ps_a.tile([65, 512], f32, tag="aT", name=f"aps{i}_{qt}_{h}")
                            for h in range(2)
                        ]
                        for kc in range(nkc):
                            j = kc - 4 * qt  # >=0 -> diagonal-overlap chunk
                            for h in range(2):
                                p0 = 64 * h
                                sT = ps_s.tile(
                                    [128, 512], f32, tag="sT", name=f"sT{i}_{qt}_{kc}_{h}"
                                )
                                nc.tensor.matmul(
                                    sT[:],
                                    qkT[p0 : p0 + 64, 4 + i, kc * 128 : (kc + 1) * 128],
                                    qkT[p0 : p0 + 64, i, qt * 512 : (qt + 1) * 512],
                                    start=True,
                                    stop=True,
                                )
                                pT = ptmp.tile(
                                    [128, 512], bf16, tag="pT", name=f"pT{i}_{qt}_{kc}_{h}"
                                )
                                nc.scalar.activation(
                                    pT[:],
                                    sT[:],
                                    mybir.ActivationFunctionType.Exp,
                                    bias=zb[:],
                                    scale=0.125,
                                )
                                if j >= 0:
                                    nc.vector.tensor_tensor(
                                        pT[:],
                                        pT[:],
                                        msk_sb[:, j, :],
                                        mybir.AluOpType.mult,
                                    )
                                nc.tensor.matmul(
                                    a_ps[h][:],
                                    von[:, kc, 2 * i + h, :],
                                    pT[:],
                                    start=(kc == 0),
                                    stop=(kc == nkc - 1),
                                )
                        for h in range(2):
                            rec = ptmp.tile([1, 512], f32, tag="rec")
                            nc.vector.reciprocal(rec[:], a_ps[h][64:65, :])
                            rb = ps_r.tile([64, 512], f32, tag="rb")
                            nc.tensor.matmul(
                                rb[:], ones1[:], rec[:], start=True, stop=True
                            )
                            rb_sb = ptmp.tile([64, 512], f32, tag="rbs")
                            nc.vector.tensor_copy(rb_sb[:], rb[:])
                            nc.vector.tensor_tensor(
                                aT[64 * h : 64 * h + 64, i, qt * 512 : (qt + 1) * 512],
                                a_ps[h][0:64, :],
                                rb_sb[:],
                                mybir.AluOpType.mult,
                            )

                for i in range(4):
                    gemm1a_chunk(i)
                    gemm1a_chunk(4 + i)
                    flash_pair(i)

            # ---- AllGather of aT across the 2-core batch pair
            nc.sync.dma_start(
                ag_in[:].rearrange("(fc p) t -> p fc t", p=128), aT[:]
            )
            if sim_single:
                # timing proxy: same DRAM traffic shape as the 2-core AllGather
                nc.sync.dma_start(ag_out[0:FQK, :], ag_in[:])
                nc.sync.dma_start(ag_out[FQK:NX, :], ag_in[:])
            else:
                nc.gpsimd.collective_compute(
                    "AllGather",
                    mybir.AluOpType.bypass,
                    replica_groups=groups,
                    ins=[ag_in[:].opt()],
                    outs=[ag_out[:].opt()],
                )

            with (
                tc.tile_pool(name="g2", bufs=1) as g2p,
                tc.tile_pool(name="ps_g2", bufs=2, space="PSUM") as ps_g2,
            ):
                gath = g2p.tile([128, 8, S], bf16)
                nc.sync.dma_start(
                    gath[:], ag_out[:].rearrange("(c p) t -> p c t", p=128)
                )
                for tt in range(TT16):
                    ps = ps_g2.tile([128, 512], f32, tag="g2")
                    for c in range(8):
                        nc.tensor.matmul(
                            ps[:],
                            gath[:, c, tt * 128 : (tt + 1) * 128],
                            wpj_sb[:, c, :],
                            start=(c == 0),
                            stop=(c == 7),
                        )
                    og = oev.tile([128, 512], f32, tag="og")
                    nc.vector.tensor_tensor(
                        og[:], ps[:], bpj_sb[:], mybir.AluOpType.add
                    )
                    nc.sync.dma_start(out[tt * 128 : (tt + 1) * 128, :], og[:])

            if debug_taps:
                tq = oev.tile([128, 8, S], f32, tag="tapq")
                nc.vector.tensor_copy(tq[:], qkT[:])
                nc.sync.dma_start(qkT_tap[:], tq[:])
                tv = oev.tile([128, TT16, HPC, 65], f32, tag="tapv")
                nc.vector.tensor_copy(tv[:], von[:])
                nc.sync.dma_start(von_tap[:], tv[:])
                ta = oev.tile([128, 4, S], f32, tag="tapa")
                nc.vector.tensor_copy(ta[:], aT[:])
                nc.sync.dma_start(aT_tap[:], ta[:])

    nc.compile()
    _BUILD_CACHE[key] = nc
    return nc


def make_in_maps(x, w_attn, b_attn, w_proj, b_proj):
    """Shard the full inputs into 8 per-core input maps."""
    x = np.asarray(x, dtype=np.float32)
    w_attn = np.asarray(w_attn, dtype=np.float32)
    b_attn = np.asarray(b_attn, dtype=np.float32)
    w_proj = np.asarray(w_proj, dtype=np.float32)
    b_proj = np.asarray(b_proj, dtype=np.float32)

    kp = np.arange(128)[:, None, None]
    jj = np.arange(4)[None, :, None]
    qf = np.arange(512)[None, None, :]
    mask = (kp + 128 * jj <= qf).astype(BF16)

    in_maps = []
    for c in range(N_CORES):
        b, g = c // 2, c % 2
        sl = slice(g * FQK, (g + 1) * FQK)
        wq = w_attn[:, 0 * NX :][:, sl]
        wk = w_attn[:, 1 * NX :][:, sl]
        wv_ = w_attn[:, 2 * NX :][:, sl]
        bq = b_attn[0 * NX :][sl]
        bk = b_attn[1 * NX :][sl]
        bv_ = b_attn[2 * NX :][sl]
        in_maps.append(
            {
                "xT": np.ascontiguousarray(x[b].T).astype(BF16),
                "wqk": np.ascontiguousarray(
                    np.concatenate([wq, wk], axis=1)
                ).astype(BF16),
                "wv": np.ascontiguousarray(wv_).astype(BF16),
                "bqk": np.ascontiguousarray(
                    np.concatenate([bq, bk]).reshape(8, 128).T
                ).astype(np.float32),
                "bvb": np.ascontiguousarray(
                    np.broadcast_to(bv_[None, :], (128, FQK))
                ).astype(np.float32),
                "wpj": np.ascontiguousarray(w_proj[:, sl]).astype(BF16),
                "bpj": np.ascontiguousarray(
                    np.broadcast_to(b_proj[None, sl], (128, FQK))
                ).astype(np.float32),
                "msk": mask,
            }
        )
    return in_maps


def assemble_out(results):
    out = np.empty((B, S, NX), dtype=np.float32)
    for c in range(N_CORES):
        b, g = c // 2, c % 2
        out[b, :, g * FQK : (g + 1) * FQK] = results[c]["out"]
    return out


def kernel(x, w_attn, b_attn, w_proj, b_proj):
    nc = build_nc()
    in_maps = make_in_maps(x, w_attn, b_attn, w_proj, b_proj)
    res = run_bass_kernel_spmd(nc, in_maps, core_ids=list(range(N_CORES)))
    return assemble_out(res.results)
